# revision 24
# baseline (speedup 1.0000x reference)
"""Trainium2 Bass kernel for IntraFrameNet (self-attention + conv head).

Math (per sample b):
  f = curr_features[b].reshape(C, M)                      # C=128, M=4096
  S = f^T f * C^-0.5   (symmetric, [M, M])
  P = softmax(S, axis=-1)
  feats1 = f @ P^T ([C, M]);  x = [feats1; f]             # [2C, M]
  y = W1 @ x + b1 -> BN(inference) -> leaky_relu(0.01)
  pred = w2 @ y + b2                                      # [1, M]

Device strategy (data-parallel, 1 sample / core, 8 cores):
  ACT is the bottleneck (128 exp tiles of [128,1024] at ~1038ns), so every
  other engine is kept off ACT's critical path:
  - Softmax denominators D ride the PE: since E=exp(S) is symmetric, row
    sums equal column sums, and a tile's column sums are a ones-matmul
    (out [128,1], engine cost ~0.4ns) accumulated in PSUM across the
    chunk loop.  No DVE tensor_reduce, no ACT accum_out.
  - D for m-super s completes when super s's chunk loop ends, so group
    s's whole head pipeline (Dinv, broadcast, fnorm, conv, pred) runs
    interleaved in super s+1's window; only group 3's head is in the
    tail.
  - S chunks [n-chunk, m-super] by PE with f in bf16; PV accumulates
    ot[c, m_super] += matmul(lhsT=fT[chunk], rhs=exp tile).
  - Head: Dinv broadcast via PE transpose + sel-matmul; conv1 with BN
    folded on host; leaky relu on DVE in-loop / fused ACT Prelu for the
    tail group (ACT is idle there).
"""

import numpy as np
import ml_dtypes

import concourse.bass as bass
from concourse import bacc
import concourse.mybir as mybir
import concourse.tile as tile
from concourse.bass_utils import run_bass_kernel_spmd
from concourse.masks import make_identity

B, C, H, W = 8, 128, 64, 64
M = H * W          # 4096
NCH = M // 128     # 32 chunks of n
SUP = 1024         # m columns per super-block
NSUP = M // SUP    # 4
CPS = SUP // 128   # 8 chunks per super
SCALE = float(C) ** -0.5
BN_EPS = 1e-5
LEAKY = 0.01

f32 = mybir.dt.float32
f32r = mybir.dt.float32r
bf16 = mybir.dt.bfloat16
AF = mybir.ActivationFunctionType
AX = mybir.AxisListType
OP = mybir.AluOpType

DEBUG_D = False  # dump the Dps accumulator to a debug output


def _build():
    nc = bacc.Bacc("TRN2", target_bir_lowering=False)

    f_d = nc.dram_tensor("f", [C, M], bf16, kind="ExternalInput")
    fT_d = nc.dram_tensor("fT", [128, NCH * 128], bf16, kind="ExternalInput")
    w1aT_d = nc.dram_tensor("w1aT", [C, C], f32r, kind="ExternalInput")
    w1bT_d = nc.dram_tensor("w1bT", [C, C], bf16, kind="ExternalInput")
    bhead_d = nc.dram_tensor("bhead", [C, 1], f32, kind="ExternalInput")
    w2T_d = nc.dram_tensor("w2T", [C, 1], f32r, kind="ExternalInput")
    sel_d = nc.dram_tensor("sel", [CPS, CPS * 128], f32r, kind="ExternalInput")
    pred_d = nc.dram_tensor("pred", [1, M], f32, kind="ExternalOutput")
    if DEBUG_D:
        dbgD_d = nc.dram_tensor("dbgD", [128, NSUP * CPS], f32,
                                kind="ExternalOutput")

    with tile.TileContext(nc) as tc:
        with (
            tc.tile_pool(name="singles", bufs=1) as singles,
            tc.tile_pool(name="pbufp", bufs=40) as pbufp,
            tc.tile_pool(name="sbm", bufs=3) as sbm,
            tc.tile_pool(name="ps_s", bufs=2, space="PSUM") as ps_s,
            tc.tile_pool(name="ps_o", bufs=1, space="PSUM") as ps_o,
            tc.tile_pool(name="ps_h", bufs=1, space="PSUM") as ps_h,
            tc.tile_pool(name="ps_d", bufs=1, space="PSUM") as ps_d,
        ):
            # ---- load inputs; f and fT pieces interleaved by first use ----
            fb = singles.tile([C, M], bf16)
            fT = singles.tile([128, NCH, 128], bf16)  # [n_local, chunk, c]
            fT_flat = fT.rearrange("p a b -> p (a b)")

            def f_piece(q):
                nc.sync.dma_start(
                    out=fb[:, q * 512 : (q + 1) * 512],
                    in_=f_d[:, q * 512 : (q + 1) * 512],
                )

            def ft_piece(q):
                nc.sync.dma_start(
                    out=fT_flat[:, q * 512 : (q + 1) * 512],
                    in_=fT_d[:, q * 512 : (q + 1) * 512],
                )

            # piece 0 on the SP ring; piece 1 and the first fT piece on the
            # Activation ring (idle until the first S lands) so the two
            # pieces gating the first exp arrive in parallel
            f_piece(0)
            nc.scalar.dma_start(
                out=fb[:, 512:1024], in_=f_d[:, 512:1024]
            )
            nc.scalar.dma_start(
                out=fT_flat[:, 0:512], in_=fT_d[:, 0:512]
            )
            for q in [2, 3]:
                f_piece(q)
            ft_piece(1)
            for q in [4, 5]:
                f_piece(q)
            ft_piece(2)
            for q in [6, 7]:
                f_piece(q)
            for q in range(3, 8):
                ft_piece(q)
            w1aT = singles.tile([C, C], f32r)
            nc.sync.dma_start(out=w1aT, in_=w1aT_d[:, :])
            w1bT = singles.tile([C, C], bf16)
            nc.sync.dma_start(out=w1bT, in_=w1bT_d[:, :])
            bhead = singles.tile([C, 1], f32)
            nc.sync.dma_start(out=bhead, in_=bhead_d[:, :])
            w2T = singles.tile([C, 1], f32r)
            nc.sync.dma_start(out=w2T, in_=w2T_d[:, :])
            sel = singles.tile([CPS, CPS * 128], f32r)
            nc.sync.dma_start(out=sel, in_=sel_d[:, :])

            # ---- identity for the Dinv transpose; ones for the D matmuls ----
            ident_f32 = singles.tile([128, 128], f32)
            make_identity(nc, ident_f32)
            ones_sb = singles.tile([128, 1], bf16)
            nc.vector.memset(ones_sb, 1.0)

            # D[m] accumulator for groups 0..2: column (s*CPS + j) =
            # denominator for m-block j of super s, accumulated over all 32
            # n-chunks by PE.  A start=True matmul zeroes its whole PSUM
            # bank, so the bank is memset once and every D matmul
            # accumulates (start=False).
            Dps = ps_d.tile([128, 3 * CPS], f32)
            nc.vector.memset(Dps, 0.0)
            # group 3's D instead rides DVE rowsum partials of chunks 24..31
            # (row sums == column sums by symmetry), so that its Dinv and
            # broadcast finish in-loop: super 3 runs its chunks in the order
            # [24..31, 0..23] and only the fnorm waits on the final PV.
            pD3 = singles.tile([128, CPS, NSUP], f32)

            O_sb = singles.tile([C, 3 * SUP], f32r)
            pred_sb = singles.tile([1, M], f32)

            drow_tiles = {}

            def head_d(g):
                """Dinv for m-group g straight off the PSUM accumulator."""
                Dinvg = sbm.tile([128, CPS], f32, tag="Dinvg", name=f"Dinvg{g}")
                nc.vector.reciprocal(
                    out=Dinvg, in_=Dps[:, CPS * g : CPS * (g + 1)]
                )
                drow_tiles[g] = Dinvg

            def head_d3():
                """Dinv for m-group 3 from the DVE rowsum partials."""
                Dg = sbm.tile([128, CPS], f32, tag="Dinvg", name="Dg3")
                nc.vector.tensor_reduce(out=Dg, in_=pD3, axis=AX.X, op=OP.add)
                Dinvg = sbm.tile([128, CPS], f32, tag="Dinvg", name="Dinvg3")
                nc.vector.reciprocal(out=Dinvg, in_=Dg)
                drow_tiles[3] = Dinvg

            drpg_tiles = {}

            def head_t_pe(g, pool, ptag):
                """Transpose Dinv for m-group g (PE half)."""
                Dinvg = drow_tiles[g]
                drpg = pool.tile([CPS, 128], f32, tag=ptag, name=f"drpg{g}")
                nc.tensor.transpose(drpg, Dinvg, ident_f32)
                drpg_tiles[g] = drpg

            def head_t_copy(g):
                """Transpose Dinv for m-group g (DVE copy half)."""
                drpg = drpg_tiles.pop(g)
                DrowTg = sbm.tile([CPS, 128], f32r, tag="DrowTg", name=f"DrowTg{g}")
                nc.vector.tensor_copy(out=DrowTg, in_=drpg)
                drow_tiles[g] = DrowTg

            fnorm_tiles = {}
            dinvb_tiles = {}
            dbp_tiles = {}
            yp_tiles = {}

            def bcast_half(g, h, half, pool, ptag):
                """Dinv broadcast [128,512] via PE sel-matmuls, 2 per call so
                no head bunch ever delays the S stream in the PE FIFO."""
                if half == 0:
                    dbp_tiles[(g, h)] = pool.tile(
                        [128, 512], f32, tag=ptag, name=f"dbp{g}_{h}"
                    )
                dbp = dbp_tiles[(g, h)]
                DrowTg = drow_tiles[g]
                for j in (2 * half, 2 * half + 1):
                    jj = h * 4 + j
                    nc.tensor.matmul(
                        dbp[:, j * 128 : (j + 1) * 128],
                        lhsT=sel[:, jj * 128 : (jj + 1) * 128],
                        rhs=DrowTg,
                        start=True,
                        stop=True,
                    )

            def head_dinvb(g, h):
                """Stage the Dinv broadcast in SBUF (tail group: fnorm then
                only waits on the final PV)."""
                dinvb = sbm.tile([128, 512], f32, tag="dinvb", name=f"dvb{h}")
                nc.vector.tensor_copy(out=dinvb, in_=dbp_tiles.pop((g, h)))
                dinvb_tiles[(g, h)] = dinvb

            def head_fnorm3(g, h):
                """Tail group: fnorm straight off the ot PSUM tile."""
                src0 = ot_tiles[g][:, h * 512 : (h + 1) * 512]
                fnorm = sbm.tile([128, 512], f32r, tag="fnorm", name=f"fn{g}_{h}")
                nc.vector.tensor_tensor(
                    out=fnorm, in0=src0, in1=dinvb_tiles.pop((g, h)), op=OP.mult
                )
                fnorm_tiles[(g, h)] = fnorm

            def head_fnorm(g, h):
                """In-loop groups: fnorm from O_sb."""
                base = g * SUP + h * 512
                fnorm = sbm.tile([128, 512], f32r, tag="fnorm", name=f"fn{g}_{h}")
                nc.vector.tensor_tensor(
                    out=fnorm, in0=O_sb[:, base : base + 512],
                    in1=dbp_tiles.pop((g, h)), op=OP.mult,
                )
                fnorm_tiles[(g, h)] = fnorm

            zsb_tiles = {}

            def head_yp(g, h, k, pool, ptag):
                """First conv, one matmul per call."""
                if k == 0:
                    yp_tiles[(g, h)] = pool.tile(
                        [128, 512], f32, tag=ptag, name=f"yp{g}_{h}"
                    )
                    nc.tensor.matmul(
                        yp_tiles[(g, h)], lhsT=w1aT,
                        rhs=fnorm_tiles.pop((g, h)), start=True, stop=False,
                    )
                else:
                    base = g * SUP + h * 512
                    nc.tensor.matmul(
                        yp_tiles[(g, h)], lhsT=w1bT,
                        rhs=fb[:, bass.ds(base, 512)], start=False, stop=True,
                    )

            def head_leaky(g, h):
                """Bias + leaky relu on DVE (in-loop: ACT is the bottleneck)."""
                yp = yp_tiles.pop((g, h))
                zsb = sbm.tile([128, 512], f32r, tag="zsb", name=f"zsb{g}_{h}")
                t1 = sbm.tile([128, 512], f32, tag="t1", name=f"t1_{g}_{h}")
                nc.vector.tensor_scalar_add(out=t1, in0=yp, scalar1=bhead)
                nc.vector.scalar_tensor_tensor(
                    out=zsb, in0=t1, scalar=LEAKY, in1=t1,
                    op0=OP.mult, op1=OP.max,
                )
                zsb_tiles[(g, h)] = zsb

            def head_mid3(g, h, pool, ptag):
                """Tail group: conv + fused ACT Prelu (ACT is idle there)."""
                head_yp(g, h, 0, pool, ptag)
                head_yp(g, h, 1, pool, ptag)
                yp = yp_tiles.pop((g, h))
                zsb = sbm.tile([128, 512], f32r, tag="zsb", name=f"zsb{g}_{h}")
                nc.scalar.activation(
                    out=zsb, in_=yp, func=AF.Prelu, bias=bhead, scale=1.0,
                    alpha=LEAKY,
                )
                zsb_tiles[(g, h)] = zsb

            def head_post(g, h, pool, ptag):
                """Final 1-channel conv + pred copy + piecewise DMA out."""
                base = g * SUP + h * 512
                hsl = bass.ds(base, 512)
                pp = pool.tile([1, 512], f32, tag=ptag, name=f"pp{g}_{h}")
                nc.tensor.matmul(
                    pp, lhsT=w2T, rhs=zsb_tiles.pop((g, h)), start=True, stop=True
                )
                if g == 3:
                    # tail: ACT is idle; Prelu(alpha=1) is an ACT copy that
                    # stays in the already-loaded table set
                    nc.scalar.activation(
                        out=pred_sb[0:1, hsl], in_=pp, func=AF.Prelu, scale=1.0,
                        alpha=1.0,
                    )
                    if h == 1:  # one merged DMA for the whole tail group
                        nc.sync.dma_start(
                            out=pred_d[:, g * SUP : (g + 1) * SUP],
                            in_=pred_sb[0:1, g * SUP : (g + 1) * SUP],
                        )
                else:
                    nc.vector.tensor_copy(out=pred_sb[0:1, hsl], in_=pp)
                    nc.sync.dma_start(out=pred_d[:, base : base + 512],
                                      in_=pred_sb[0:1, hsl])

            # ---- main attention loop ----
            # Global chunk stream with 1-chunk S-matmul lookahead so an
            # eviction-stalled PV never blocks the next S (and hence exp).
            s3_order = list(range(3 * CPS, NCH)) + list(range(0, 3 * CPS))
            seq = [(s, t) for s in range(NSUP - 1) for t in range(NCH)]
            seq += [(NSUP - 1, t) for t in s3_order]
            st_tiles = {}
            ot_tiles = {}

            def emit_s(idx):
                s, t = seq[idx]
                st = ps_s.tile([128, SUP], f32, tag="st", name=f"st{s}_{t}")
                for q in range(2):
                    nc.tensor.matmul(
                        st[:, q * 512 : (q + 1) * 512],
                        lhsT=fb[:, t * 128 : (t + 1) * 128],
                        rhs=fb[:, s * SUP + q * 512 : s * SUP + (q + 1) * 512],
                        start=True,
                        stop=True,
                    )
                st_tiles[(s, t)] = st

            def emit_pv(s, t, p, pb):
                """PV + D ones-matmuls for chunk (s,t); lags the exp stream by
                one chunk so the next chunk's S-matmuls reach the head of the
                PE queue the moment their st slot frees (keeps ACT fed)."""
                if p == 0:
                    ot_tiles[s] = ps_o.tile([C, SUP], f32, tag="ot", name=f"ot{s}")
                ot = ot_tiles[s]
                for q in range(2):
                    nc.tensor.matmul(
                        ot[:, q * 512 : (q + 1) * 512],
                        lhsT=fT[:, t, :],
                        rhs=pb[:, q * 512 : (q + 1) * 512],
                        start=(p == 0),
                        stop=(p == NCH - 1),
                    )
                if s < NSUP - 1:
                    for j in range(CPS):
                        nc.tensor.matmul(
                            Dps[:, s * CPS + j : s * CPS + j + 1],
                            lhsT=pb[:, j * 128 : (j + 1) * 128],
                            rhs=ones_sb,
                            start=False,
                            stop=(p == NCH - 1),
                        )
                if p == NCH - 1 and s < NSUP - 1:
                    for q in range(2):
                        nc.vector.tensor_copy(
                            out=O_sb[:, s * SUP + q * 512 : s * SUP + (q + 1) * 512],
                            in_=ot[:, q * 512 : (q + 1) * 512],
                        )

            emit_s(0)
            pv_pending = None
            pb_hold = {}
            for i, (s, t) in enumerate(seq):
                p = i % NCH  # position within the super
                st = st_tiles.pop((s, t))
                pb = pbufp.tile([128, SUP], bf16, tag="pb", name=f"pb{s}_{t}")
                if s == NSUP - 1 and t >= 3 * CPS:
                    # super 3 runs chunks 24..31 first, where their DVE
                    # reduces would collide with the head consumers that
                    # hold the ps_h bank -- ride the exp's accumulator
                    nc.scalar.activation(
                        out=pb, in_=st, func=AF.Exp, scale=SCALE,
                        accum_out=pD3[:, t - 3 * CPS, s : s + 1],
                    )
                elif i == 0:
                    # first tile: exp in two halves so the first half only
                    # waits on f piece 0 (the halves' S matmuls land in order)
                    for q in range(2):
                        nc.scalar.activation(
                            out=pb[:, q * 512 : (q + 1) * 512],
                            in_=st[:, q * 512 : (q + 1) * 512],
                            func=AF.Exp, scale=SCALE,
                        )
                else:
                    nc.scalar.activation(out=pb, in_=st, func=AF.Exp, scale=SCALE)
                if i + 1 < len(seq):
                    emit_s(i + 1)
                if pv_pending is not None:
                    emit_pv(*pv_pending)
                pv_pending = (s, t, p, pb)
                if t >= NCH - 2 and s < NSUP - 1:
                    pb_hold[t] = pb
                if 3 * CPS <= t < NCH - 2 and s < NSUP - 1:
                    # rowsum partial for group 3's D
                    nc.vector.tensor_reduce(
                        out=pD3[:, t - 3 * CPS, s : s + 1], in_=pb,
                        axis=AX.X, op=OP.add,
                    )
                if s >= 1 and p in (2, 3):
                    # deferred rowsum partials for the previous super's last
                    # two chunks, so the O_sb copies aren't queued behind
                    # them on DVE at the super boundary
                    td = NCH - 2 + (p - 2)
                    nc.vector.tensor_reduce(
                        out=pD3[:, td - 3 * CPS, s - 1 : s], in_=pb_hold[td],
                        axis=AX.X, op=OP.add,
                    )
                # head pipeline for the previous super, interleaved into this
                # super's window (its D and ot completed at the super switch)
                # one small instruction per position so no head bunch ever
                # sits in the PE FIFO ahead of the S stream
                if s >= 1:
                    g = s - 1
                    if p == 4:
                        head_d(g)
                    elif p == 5:
                        head_t_pe(g, ps_h, "ph")
                        head_t_copy(g)
                    elif p == 7:
                        bcast_half(g, 0, 0, ps_h, "ph")
                    elif p == 8:
                        bcast_half(g, 0, 1, ps_h, "ph")
                    elif p == 9:
                        head_fnorm(g, 0)
                    elif p == 10:
                        head_yp(g, 0, 0, ps_h, "ph")
                    elif p == 11:
                        head_yp(g, 0, 1, ps_h, "ph")
                    elif p == 12:
                        head_leaky(g, 0)
                    elif p == 13:
                        head_post(g, 0, ps_h, "ph")
                    elif p == 15:
                        bcast_half(g, 1, 0, ps_h, "ph")
                    elif p == 16:
                        bcast_half(g, 1, 1, ps_h, "ph")
                    elif p == 17:
                        head_fnorm(g, 1)
                    elif p == 18:
                        head_yp(g, 1, 0, ps_h, "ph")
                    elif p == 19:
                        head_yp(g, 1, 1, ps_h, "ph")
                    elif p == 20:
                        head_leaky(g, 1)
                    elif p == 21:
                        head_post(g, 1, ps_h, "ph")
                if s == NSUP - 1:
                    # group 3's Dinv + broadcast staging, in-loop (partials
                    # complete once chunks 24..31 -- processed first -- are
                    # done); ps_h is free of group-2 tiles after p==21
                    if p == 10:
                        head_d3()
                    elif p == 22:
                        head_t_pe(3, ps_h, "ph")
                        head_t_copy(3)
                    elif p == 24:
                        bcast_half(3, 0, 0, ps_h, "ph")
                    elif p == 25:
                        bcast_half(3, 0, 1, ps_h, "ph")
                    elif p == 26:
                        head_dinvb(3, 0)
                    elif p == 27:
                        bcast_half(3, 1, 0, ps_h, "ph")
                    elif p == 28:
                        bcast_half(3, 1, 1, ps_h, "ph")
                    elif p == 29:
                        head_dinvb(3, 1)

            # ---- tail: only group 3's fnorm onward (gated by final PV) ----
            emit_pv(*pv_pending)
            head_fnorm3(3, 0)
            head_mid3(3, 0, ps_s, "st")
            head_fnorm3(3, 1)
            head_mid3(3, 1, ps_s, "st")
            head_post(3, 0, ps_s, "st")
            head_post(3, 1, ps_s, "st")

    nc.finalize()
    return nc


_NC = None


def _get_nc():
    global _NC
    if _NC is None:
        _NC = _build()
    return _NC


def _prepare_in_maps(inputs):
    curr = np.asarray(inputs["curr_features"], np.float32)
    w1 = np.asarray(inputs["w1"], np.float32)
    b1 = np.asarray(inputs["b1"], np.float32)
    gamma = np.asarray(inputs["gamma"], np.float32)
    beta = np.asarray(inputs["beta"], np.float32)
    rm = np.asarray(inputs["running_mean"], np.float32)
    rv = np.asarray(inputs["running_var"], np.float32)
    w2 = np.asarray(inputs["w2"], np.float32)

    # fold BN (inference) into the first conv
    a = gamma / np.sqrt(rv + BN_EPS)                      # [C]
    W1f = w1 * a[:, None]                                 # [C, 2C]
    bhead = (b1 * a + beta - rm * a).astype(np.float32).reshape(C, 1)
    w1aT = np.ascontiguousarray(W1f[:, :C].T, np.float32)  # feats1 part
    w1bT = np.ascontiguousarray(W1f[:, C:].T).astype(ml_dtypes.bfloat16)
    w2T = np.ascontiguousarray(w2.T, np.float32)           # [C, 1]

    selm = np.zeros((CPS, CPS * 128), np.float32)
    for k in range(CPS):
        selm[k, k * 128 : (k + 1) * 128] = 1.0

    in_maps = []
    for b in range(B):
        in_maps.append(
            {
                "f": np.ascontiguousarray(curr[b].reshape(C, M)).astype(
                    ml_dtypes.bfloat16
                ),
                "fT": np.ascontiguousarray(
                    curr[b].reshape(C, NCH, 128).transpose(2, 1, 0).reshape(
                        128, NCH * 128
                    )
                ).astype(ml_dtypes.bfloat16),
                "w1aT": w1aT,
                "w1bT": w1bT,
                "bhead": bhead,
                "w2T": w2T,
                "sel": selm,
            }
        )
    return in_maps


def kernel(**inputs):
    b2 = np.asarray(inputs["b2"], np.float32)
    nc = _get_nc()
    in_maps = _prepare_in_maps(inputs)
    res = run_bass_kernel_spmd(nc, in_maps, core_ids=list(range(B)))
    preds = np.stack([r["pred"].reshape(1, H, W) for r in res.results], axis=0)
    return (preds + b2[0]).astype(np.float32)


if __name__ == "__main__":
    _build()
    print("build OK")


# revision 25
# speedup vs baseline: 1.0036x; 1.0036x over previous
"""Trainium2 Bass kernel for IntraFrameNet (self-attention + conv head).

Math (per sample b):
  f = curr_features[b].reshape(C, M)                      # C=128, M=4096
  S = f^T f * C^-0.5   (symmetric, [M, M])
  P = softmax(S, axis=-1)
  feats1 = f @ P^T ([C, M]);  x = [feats1; f]             # [2C, M]
  y = W1 @ x + b1 -> BN(inference) -> leaky_relu(0.01)
  pred = w2 @ y + b2                                      # [1, M]

Device strategy (data-parallel, 1 sample / core, 8 cores):
  ACT is the bottleneck (128 exp tiles of [128,1024] at ~1038ns), so every
  other engine is kept off ACT's critical path:
  - Softmax denominators D ride the PE: since E=exp(S) is symmetric, row
    sums equal column sums, and a tile's column sums are a ones-matmul
    (out [128,1], engine cost ~0.4ns) accumulated in PSUM across the
    chunk loop.  No DVE tensor_reduce, no ACT accum_out.
  - D for m-super s completes when super s's chunk loop ends, so group
    s's whole head pipeline (Dinv, broadcast, fnorm, conv, pred) runs
    interleaved in super s+1's window; only group 3's head is in the
    tail.
  - S chunks [n-chunk, m-super] by PE with f in bf16; PV accumulates
    ot[c, m_super] += matmul(lhsT=fT[chunk], rhs=exp tile).
  - Head: Dinv broadcast via PE transpose + sel-matmul; conv1 with BN
    folded on host; leaky relu on DVE in-loop / fused ACT Prelu for the
    tail group (ACT is idle there).
"""

import numpy as np
import ml_dtypes

import concourse.bass as bass
from concourse import bacc
import concourse.mybir as mybir
import concourse.tile as tile
from concourse.bass_utils import run_bass_kernel_spmd
from concourse.masks import make_identity

B, C, H, W = 8, 128, 64, 64
M = H * W          # 4096
NCH = M // 128     # 32 chunks of n
SUP = 1024         # m columns per super-block
NSUP = M // SUP    # 4
CPS = SUP // 128   # 8 chunks per super
SCALE = float(C) ** -0.5
BN_EPS = 1e-5
LEAKY = 0.01

f32 = mybir.dt.float32
f32r = mybir.dt.float32r
bf16 = mybir.dt.bfloat16
AF = mybir.ActivationFunctionType
AX = mybir.AxisListType
OP = mybir.AluOpType

DEBUG_D = False  # dump the Dps accumulator to a debug output


def _build():
    nc = bacc.Bacc("TRN2", target_bir_lowering=False)

    f_d = nc.dram_tensor("f", [C, M], bf16, kind="ExternalInput")
    fT_d = nc.dram_tensor("fT", [128, NCH * 128], bf16, kind="ExternalInput")
    w1aT_d = nc.dram_tensor("w1aT", [C, C], f32r, kind="ExternalInput")
    w1bT_d = nc.dram_tensor("w1bT", [C, C], bf16, kind="ExternalInput")
    bhead_d = nc.dram_tensor("bhead", [C, 1], f32, kind="ExternalInput")
    w2T_d = nc.dram_tensor("w2T", [C, 1], f32r, kind="ExternalInput")
    sel_d = nc.dram_tensor("sel", [CPS, CPS * 128], f32r, kind="ExternalInput")
    pred_d = nc.dram_tensor("pred", [1, M], f32, kind="ExternalOutput")
    if DEBUG_D:
        dbgD_d = nc.dram_tensor("dbgD", [128, NSUP * CPS], f32,
                                kind="ExternalOutput")

    with tile.TileContext(nc) as tc:
        with (
            tc.tile_pool(name="singles", bufs=1) as singles,
            tc.tile_pool(name="pbufp", bufs=40) as pbufp,
            tc.tile_pool(name="sbm", bufs=3) as sbm,
            tc.tile_pool(name="ps_s", bufs=2, space="PSUM") as ps_s,
            tc.tile_pool(name="ps_o", bufs=1, space="PSUM") as ps_o,
            tc.tile_pool(name="ps_h", bufs=1, space="PSUM") as ps_h,
            tc.tile_pool(name="ps_d", bufs=1, space="PSUM") as ps_d,
        ):
            # ---- load inputs; f and fT pieces interleaved by first use ----
            fb = singles.tile([C, M], bf16)
            fT = singles.tile([128, NCH, 128], bf16)  # [n_local, chunk, c]
            fT_flat = fT.rearrange("p a b -> p (a b)")

            def f_piece(q):
                nc.sync.dma_start(
                    out=fb[:, q * 512 : (q + 1) * 512],
                    in_=f_d[:, q * 512 : (q + 1) * 512],
                )

            def ft_piece(q):
                nc.sync.dma_start(
                    out=fT_flat[:, q * 512 : (q + 1) * 512],
                    in_=fT_d[:, q * 512 : (q + 1) * 512],
                )

            for q in [0, 1]:
                f_piece(q)
            ft_piece(0)
            for q in [2, 3]:
                f_piece(q)
            ft_piece(1)
            for q in [4, 5]:
                f_piece(q)
            ft_piece(2)
            for q in [6, 7]:
                f_piece(q)
            for q in range(3, 8):
                ft_piece(q)
            w1aT = singles.tile([C, C], f32r)
            nc.sync.dma_start(out=w1aT, in_=w1aT_d[:, :])
            w1bT = singles.tile([C, C], bf16)
            nc.sync.dma_start(out=w1bT, in_=w1bT_d[:, :])
            bhead = singles.tile([C, 1], f32)
            nc.sync.dma_start(out=bhead, in_=bhead_d[:, :])
            w2T = singles.tile([C, 1], f32r)
            nc.sync.dma_start(out=w2T, in_=w2T_d[:, :])
            sel = singles.tile([CPS, CPS * 128], f32r)
            nc.sync.dma_start(out=sel, in_=sel_d[:, :])

            # ---- identity for the Dinv transpose; ones for the D matmuls ----
            ident_f32 = singles.tile([128, 128], f32)
            make_identity(nc, ident_f32)
            ones_sb = singles.tile([128, 1], bf16)
            nc.vector.memset(ones_sb, 1.0)

            # D[m] accumulator for groups 0..2: column (s*CPS + j) =
            # denominator for m-block j of super s, accumulated over all 32
            # n-chunks by PE.  A start=True matmul zeroes its whole PSUM
            # bank, so the bank is memset once and every D matmul
            # accumulates (start=False).
            Dps = ps_d.tile([128, 3 * CPS], f32)
            nc.vector.memset(Dps, 0.0)
            # group 3's D instead rides DVE rowsum partials of chunks 24..31
            # (row sums == column sums by symmetry), so that its Dinv and
            # broadcast finish in-loop: super 3 runs its chunks in the order
            # [24..31, 0..23] and only the fnorm waits on the final PV.
            pD3 = singles.tile([128, CPS, NSUP], f32)

            O_sb = singles.tile([C, 3 * SUP], f32r)
            pred_sb = singles.tile([1, M], f32)

            drow_tiles = {}

            def head_d(g):
                """Dinv for m-group g straight off the PSUM accumulator."""
                Dinvg = sbm.tile([128, CPS], f32, tag="Dinvg", name=f"Dinvg{g}")
                nc.vector.reciprocal(
                    out=Dinvg, in_=Dps[:, CPS * g : CPS * (g + 1)]
                )
                drow_tiles[g] = Dinvg

            def head_d3():
                """Dinv for m-group 3 from the DVE rowsum partials."""
                Dg = sbm.tile([128, CPS], f32, tag="Dinvg", name="Dg3")
                nc.vector.tensor_reduce(out=Dg, in_=pD3, axis=AX.X, op=OP.add)
                Dinvg = sbm.tile([128, CPS], f32, tag="Dinvg", name="Dinvg3")
                nc.vector.reciprocal(out=Dinvg, in_=Dg)
                drow_tiles[3] = Dinvg

            drpg_tiles = {}

            def head_t_pe(g, pool, ptag):
                """Transpose Dinv for m-group g (PE half)."""
                Dinvg = drow_tiles[g]
                drpg = pool.tile([CPS, 128], f32, tag=ptag, name=f"drpg{g}")
                nc.tensor.transpose(drpg, Dinvg, ident_f32)
                drpg_tiles[g] = drpg

            def head_t_copy(g):
                """Transpose Dinv for m-group g (DVE copy half)."""
                drpg = drpg_tiles.pop(g)
                DrowTg = sbm.tile([CPS, 128], f32r, tag="DrowTg", name=f"DrowTg{g}")
                nc.vector.tensor_copy(out=DrowTg, in_=drpg)
                drow_tiles[g] = DrowTg

            fnorm_tiles = {}
            dinvb_tiles = {}
            dbp_tiles = {}
            yp_tiles = {}

            def bcast_half(g, h, half, pool, ptag):
                """Dinv broadcast [128,512] via PE sel-matmuls, 2 per call so
                no head bunch ever delays the S stream in the PE FIFO."""
                if half == 0:
                    dbp_tiles[(g, h)] = pool.tile(
                        [128, 512], f32, tag=ptag, name=f"dbp{g}_{h}"
                    )
                dbp = dbp_tiles[(g, h)]
                DrowTg = drow_tiles[g]
                for j in (2 * half, 2 * half + 1):
                    jj = h * 4 + j
                    nc.tensor.matmul(
                        dbp[:, j * 128 : (j + 1) * 128],
                        lhsT=sel[:, jj * 128 : (jj + 1) * 128],
                        rhs=DrowTg,
                        start=True,
                        stop=True,
                    )

            def head_dinvb(g, h):
                """Stage the Dinv broadcast in SBUF (tail group: fnorm then
                only waits on the final PV)."""
                dinvb = sbm.tile([128, 512], f32, tag="dinvb", name=f"dvb{h}")
                nc.vector.tensor_copy(out=dinvb, in_=dbp_tiles.pop((g, h)))
                dinvb_tiles[(g, h)] = dinvb

            def head_fnorm3(g, h):
                """Tail group: fnorm straight off the ot PSUM tile."""
                src0 = ot_tiles[g][:, h * 512 : (h + 1) * 512]
                fnorm = sbm.tile([128, 512], f32r, tag="fnorm", name=f"fn{g}_{h}")
                nc.vector.tensor_tensor(
                    out=fnorm, in0=src0, in1=dinvb_tiles.pop((g, h)), op=OP.mult
                )
                fnorm_tiles[(g, h)] = fnorm

            def head_fnorm(g, h):
                """In-loop groups: fnorm from O_sb."""
                base = g * SUP + h * 512
                fnorm = sbm.tile([128, 512], f32r, tag="fnorm", name=f"fn{g}_{h}")
                nc.vector.tensor_tensor(
                    out=fnorm, in0=O_sb[:, base : base + 512],
                    in1=dbp_tiles.pop((g, h)), op=OP.mult,
                )
                fnorm_tiles[(g, h)] = fnorm

            zsb_tiles = {}

            def head_yp(g, h, k, pool, ptag):
                """First conv, one matmul per call."""
                if k == 0:
                    yp_tiles[(g, h)] = pool.tile(
                        [128, 512], f32, tag=ptag, name=f"yp{g}_{h}"
                    )
                    nc.tensor.matmul(
                        yp_tiles[(g, h)], lhsT=w1aT,
                        rhs=fnorm_tiles.pop((g, h)), start=True, stop=False,
                    )
                else:
                    base = g * SUP + h * 512
                    nc.tensor.matmul(
                        yp_tiles[(g, h)], lhsT=w1bT,
                        rhs=fb[:, bass.ds(base, 512)], start=False, stop=True,
                    )

            def head_leaky(g, h):
                """Bias + leaky relu on DVE (in-loop: ACT is the bottleneck)."""
                yp = yp_tiles.pop((g, h))
                zsb = sbm.tile([128, 512], f32r, tag="zsb", name=f"zsb{g}_{h}")
                t1 = sbm.tile([128, 512], f32, tag="t1", name=f"t1_{g}_{h}")
                nc.vector.tensor_scalar_add(out=t1, in0=yp, scalar1=bhead)
                nc.vector.scalar_tensor_tensor(
                    out=zsb, in0=t1, scalar=LEAKY, in1=t1,
                    op0=OP.mult, op1=OP.max,
                )
                zsb_tiles[(g, h)] = zsb

            def head_mid3(g, h, pool, ptag):
                """Tail group: conv + fused ACT Prelu (ACT is idle there)."""
                head_yp(g, h, 0, pool, ptag)
                head_yp(g, h, 1, pool, ptag)
                yp = yp_tiles.pop((g, h))
                zsb = sbm.tile([128, 512], f32r, tag="zsb", name=f"zsb{g}_{h}")
                nc.scalar.activation(
                    out=zsb, in_=yp, func=AF.Prelu, bias=bhead, scale=1.0,
                    alpha=LEAKY,
                )
                zsb_tiles[(g, h)] = zsb

            def head_post(g, h, pool, ptag):
                """Final 1-channel conv + pred copy + piecewise DMA out."""
                base = g * SUP + h * 512
                hsl = bass.ds(base, 512)
                pp = pool.tile([1, 512], f32, tag=ptag, name=f"pp{g}_{h}")
                nc.tensor.matmul(
                    pp, lhsT=w2T, rhs=zsb_tiles.pop((g, h)), start=True, stop=True
                )
                if g == 3:
                    # tail: DVE is free after the fnorms; the copy there
                    # overlaps the second half's ACT Prelu
                    nc.vector.tensor_copy(out=pred_sb[0:1, hsl], in_=pp)
                    if h == 1:  # one merged DMA for the whole tail group
                        nc.sync.dma_start(
                            out=pred_d[:, g * SUP : (g + 1) * SUP],
                            in_=pred_sb[0:1, g * SUP : (g + 1) * SUP],
                        )
                else:
                    nc.vector.tensor_copy(out=pred_sb[0:1, hsl], in_=pp)
                    nc.sync.dma_start(out=pred_d[:, base : base + 512],
                                      in_=pred_sb[0:1, hsl])

            # ---- main attention loop ----
            # Global chunk stream with 1-chunk S-matmul lookahead so an
            # eviction-stalled PV never blocks the next S (and hence exp).
            s3_order = list(range(3 * CPS, NCH)) + list(range(0, 3 * CPS))
            seq = [(s, t) for s in range(NSUP - 1) for t in range(NCH)]
            seq += [(NSUP - 1, t) for t in s3_order]
            st_tiles = {}
            ot_tiles = {}

            def emit_s(idx):
                s, t = seq[idx]
                st = ps_s.tile([128, SUP], f32, tag="st", name=f"st{s}_{t}")
                for q in range(2):
                    nc.tensor.matmul(
                        st[:, q * 512 : (q + 1) * 512],
                        lhsT=fb[:, t * 128 : (t + 1) * 128],
                        rhs=fb[:, s * SUP + q * 512 : s * SUP + (q + 1) * 512],
                        start=True,
                        stop=True,
                    )
                st_tiles[(s, t)] = st

            def emit_pv(s, t, p, pb):
                """PV + D ones-matmuls for chunk (s,t); lags the exp stream by
                one chunk so the next chunk's S-matmuls reach the head of the
                PE queue the moment their st slot frees (keeps ACT fed)."""
                if p == 0:
                    ot_tiles[s] = ps_o.tile([C, SUP], f32, tag="ot", name=f"ot{s}")
                ot = ot_tiles[s]
                for q in range(2):
                    nc.tensor.matmul(
                        ot[:, q * 512 : (q + 1) * 512],
                        lhsT=fT[:, t, :],
                        rhs=pb[:, q * 512 : (q + 1) * 512],
                        start=(p == 0),
                        stop=(p == NCH - 1),
                    )
                if s < NSUP - 1:
                    for j in range(CPS):
                        nc.tensor.matmul(
                            Dps[:, s * CPS + j : s * CPS + j + 1],
                            lhsT=pb[:, j * 128 : (j + 1) * 128],
                            rhs=ones_sb,
                            start=False,
                            stop=(p == NCH - 1),
                        )
                if p == NCH - 1 and s < NSUP - 1:
                    for q in range(2):
                        nc.vector.tensor_copy(
                            out=O_sb[:, s * SUP + q * 512 : s * SUP + (q + 1) * 512],
                            in_=ot[:, q * 512 : (q + 1) * 512],
                        )

            emit_s(0)
            pv_pending = None
            pb_hold = {}
            for i, (s, t) in enumerate(seq):
                p = i % NCH  # position within the super
                st = st_tiles.pop((s, t))
                pb = pbufp.tile([128, SUP], bf16, tag="pb", name=f"pb{s}_{t}")
                if s == NSUP - 1 and t >= 3 * CPS:
                    # super 3 runs chunks 24..31 first, where their DVE
                    # reduces would collide with the head consumers that
                    # hold the ps_h bank -- ride the exp's accumulator
                    nc.scalar.activation(
                        out=pb, in_=st, func=AF.Exp, scale=SCALE,
                        accum_out=pD3[:, t - 3 * CPS, s : s + 1],
                    )
                else:
                    nc.scalar.activation(out=pb, in_=st, func=AF.Exp, scale=SCALE)
                if i + 1 < len(seq):
                    emit_s(i + 1)
                if pv_pending is not None:
                    emit_pv(*pv_pending)
                pv_pending = (s, t, p, pb)
                if t >= NCH - 2 and s < NSUP - 1:
                    pb_hold[t] = pb
                if 3 * CPS <= t < NCH - 2 and s < NSUP - 1:
                    # rowsum partial for group 3's D
                    nc.vector.tensor_reduce(
                        out=pD3[:, t - 3 * CPS, s : s + 1], in_=pb,
                        axis=AX.X, op=OP.add,
                    )
                if s >= 1 and p in (2, 3):
                    # deferred rowsum partials for the previous super's last
                    # two chunks, so the O_sb copies aren't queued behind
                    # them on DVE at the super boundary
                    td = NCH - 2 + (p - 2)
                    nc.vector.tensor_reduce(
                        out=pD3[:, td - 3 * CPS, s - 1 : s], in_=pb_hold[td],
                        axis=AX.X, op=OP.add,
                    )
                # head pipeline for the previous super, interleaved into this
                # super's window (its D and ot completed at the super switch)
                # one small instruction per position so no head bunch ever
                # sits in the PE FIFO ahead of the S stream
                if s >= 1:
                    g = s - 1
                    if p == 4:
                        head_d(g)
                    elif p == 5:
                        head_t_pe(g, ps_h, "ph")
                        head_t_copy(g)
                    elif p == 7:
                        bcast_half(g, 0, 0, ps_h, "ph")
                    elif p == 8:
                        bcast_half(g, 0, 1, ps_h, "ph")
                    elif p == 9:
                        head_fnorm(g, 0)
                    elif p == 10:
                        head_yp(g, 0, 0, ps_h, "ph")
                    elif p == 11:
                        head_yp(g, 0, 1, ps_h, "ph")
                    elif p == 12:
                        head_leaky(g, 0)
                    elif p == 13:
                        head_post(g, 0, ps_h, "ph")
                    elif p == 15:
                        bcast_half(g, 1, 0, ps_h, "ph")
                    elif p == 16:
                        bcast_half(g, 1, 1, ps_h, "ph")
                    elif p == 17:
                        head_fnorm(g, 1)
                    elif p == 18:
                        head_yp(g, 1, 0, ps_h, "ph")
                    elif p == 19:
                        head_yp(g, 1, 1, ps_h, "ph")
                    elif p == 20:
                        head_leaky(g, 1)
                    elif p == 21:
                        head_post(g, 1, ps_h, "ph")
                if s == NSUP - 1:
                    # group 3's Dinv + broadcast staging, in-loop (partials
                    # complete once chunks 24..31 -- processed first -- are
                    # done); ps_h is free of group-2 tiles after p==21
                    if p == 10:
                        head_d3()
                    elif p == 22:
                        head_t_pe(3, ps_h, "ph")
                        head_t_copy(3)
                    elif p == 24:
                        bcast_half(3, 0, 0, ps_h, "ph")
                    elif p == 25:
                        bcast_half(3, 0, 1, ps_h, "ph")
                    elif p == 26:
                        head_dinvb(3, 0)
                    elif p == 27:
                        bcast_half(3, 1, 0, ps_h, "ph")
                    elif p == 28:
                        bcast_half(3, 1, 1, ps_h, "ph")
                    elif p == 29:
                        head_dinvb(3, 1)

            # ---- tail: only group 3's fnorm onward (gated by final PV) ----
            emit_pv(*pv_pending)
            head_fnorm3(3, 0)
            head_mid3(3, 0, ps_s, "st")
            head_fnorm3(3, 1)
            head_mid3(3, 1, ps_s, "st")
            head_post(3, 0, ps_s, "st")
            head_post(3, 1, ps_s, "st")

    nc.finalize()
    return nc


_NC = None


def _get_nc():
    global _NC
    if _NC is None:
        _NC = _build()
    return _NC


def _prepare_in_maps(inputs):
    curr = np.asarray(inputs["curr_features"], np.float32)
    w1 = np.asarray(inputs["w1"], np.float32)
    b1 = np.asarray(inputs["b1"], np.float32)
    gamma = np.asarray(inputs["gamma"], np.float32)
    beta = np.asarray(inputs["beta"], np.float32)
    rm = np.asarray(inputs["running_mean"], np.float32)
    rv = np.asarray(inputs["running_var"], np.float32)
    w2 = np.asarray(inputs["w2"], np.float32)

    # fold BN (inference) into the first conv
    a = gamma / np.sqrt(rv + BN_EPS)                      # [C]
    W1f = w1 * a[:, None]                                 # [C, 2C]
    bhead = (b1 * a + beta - rm * a).astype(np.float32).reshape(C, 1)
    w1aT = np.ascontiguousarray(W1f[:, :C].T, np.float32)  # feats1 part
    w1bT = np.ascontiguousarray(W1f[:, C:].T).astype(ml_dtypes.bfloat16)
    w2T = np.ascontiguousarray(w2.T, np.float32)           # [C, 1]

    selm = np.zeros((CPS, CPS * 128), np.float32)
    for k in range(CPS):
        selm[k, k * 128 : (k + 1) * 128] = 1.0

    in_maps = []
    for b in range(B):
        in_maps.append(
            {
                "f": np.ascontiguousarray(curr[b].reshape(C, M)).astype(
                    ml_dtypes.bfloat16
                ),
                "fT": np.ascontiguousarray(
                    curr[b].reshape(C, NCH, 128).transpose(2, 1, 0).reshape(
                        128, NCH * 128
                    )
                ).astype(ml_dtypes.bfloat16),
                "w1aT": w1aT,
                "w1bT": w1bT,
                "bhead": bhead,
                "w2T": w2T,
                "sel": selm,
            }
        )
    return in_maps


def kernel(**inputs):
    b2 = np.asarray(inputs["b2"], np.float32)
    nc = _get_nc()
    in_maps = _prepare_in_maps(inputs)
    res = run_bass_kernel_spmd(nc, in_maps, core_ids=list(range(B)))
    preds = np.stack([r["pred"].reshape(1, H, W) for r in res.results], axis=0)
    return (preds + b2[0]).astype(np.float32)


if __name__ == "__main__":
    _build()
    print("build OK")
